# revision 1
# baseline (speedup 1.0000x reference)
"""ConditionEmbedder kernel for 8 Trainium2 NeuronCores.

Math (train=0, unconditioned=0 path):
    drop = isnan(labels);  safe = where(drop, 0, labels)
    s    = softmax(safe[:,d,None]*w1[d] + b1[d], axis=-1)        # per (b, d)
    mlp  = s @ w2[d].T
    out  = sum_d where(drop, emb_w[d], mlp)                      # [B, H]

Device strategy (pure data parallel over batch, 8 cores):
    * softmax is computed WITHOUT a division:  s = exp(w1*x - lnZ(x)),
      where lnZ_d(x) = log sum_h exp(x*w1[d,h] + b1[d,h]) is approximated
      by a per-d degree-16 polynomial evaluated on the vector engine
      (b1 is folded into w2:  w2e = w2 * exp(b1)).
    * the [h, b] logits tile is produced by a K=6 matmul that simultaneously
      broadcasts x across 128 partitions, applies w1 (bf16 hi/lo split for
      fp32 accuracy), and adds -lnZ (the normalizer) and the -1e30 drop mask
      baked into the lncz rows.
    * scalar engine does one exp pass (the hard floor of this problem).
    * the H x H matmuls run as float32r (full PE rate) accumulating all 8 d
      plus the embedding-fallback term into one PSUM tile [k, b].
    * output is written transposed [H, B_core]; the host untransposes.
"""

import sys

import numpy as np

_B, _D, _H = 131072, 8, 128
_NCORES = 8
_BC = _B // _NCORES          # batch rows per core
_NSTRIPE = _BC // 512        # 512-row stripes per core
_DEG = 16                    # lnZ polynomial degree (zero-padded if fit is lower)


def _np_reference(labels, emb_w, w1, b1, w2, train, unconditioned):
    """Slow exact fallback for the train/unconditioned branches (uses jax to
    reproduce the reference PRNG streams)."""
    import jax
    import jax.numpy as jnp

    DROPOUT_PROB = 0.1
    labels = jnp.asarray(labels)
    if unconditioned:
        drop = jnp.ones(labels.shape, dtype=bool)
    else:
        drop = jnp.isnan(labels)
        if train:
            rkey = jax.random.fold_in(jax.random.key(0), 1)
            drop = drop | (jax.random.uniform(rkey, labels.shape) < DROPOUT_PROB)
    safe = jnp.where(drop, 0.0, labels)
    h1 = safe[:, :, None] * w1[None, :, :] + b1[None, :, :]
    s = jax.nn.softmax(h1, axis=-1)
    mlp = jnp.einsum('bdh,dkh->bdk', s, w2)
    emb = jnp.where(drop[:, :, None], emb_w[None, :, :], mlp)
    if train:
        nkey = jax.random.fold_in(jax.random.key(0), 2)
        emb = emb + jax.random.normal(nkey, emb.shape, dtype=emb.dtype)
    return np.asarray(emb.sum(axis=1))


def _fit_lnz_coeffs(w1, b1, S):
    """Per-d monomial coefficients c[d, 0..DEG] with poly(u) ~= -lnZ_d(S*u),
    validated against a simulated fp32 reversed-Horner evaluation."""
    import numpy.polynomial.chebyshev as CH

    G = 8193
    u = np.linspace(-1.0, 1.0, G)
    x = S * u
    lg = x[:, None, None] * w1[None].astype(np.float64) + b1[None].astype(np.float64)
    m = lg.max(-1)
    lnZ = m + np.log(np.exp(lg - m[..., None]).sum(-1))
    target = -lnZ  # [G, 8]

    uf = u.astype(np.float32)
    coeffs = np.zeros((_D, _DEG + 1), np.float64)
    for d in range(_D):
        best = None
        for deg in range(8, _DEG + 1):
            cf = CH.chebfit(u, target[:, d], deg)
            pc = CH.cheb2poly(cf)
            cc = np.zeros(_DEG + 1)
            cc[: len(pc)] = pc
            acc = np.zeros(G, np.float32)
            for k in range(_DEG, 0, -1):
                acc = ((acc + np.float32(cc[k])) * uf).astype(np.float32)
            err = np.abs((acc + np.float32(cc[0])).astype(np.float32) - target[:, d]).max()
            if best is None or err < best[0]:
                best = (err, cc)
        coeffs[d] = best[1]
    return coeffs.astype(np.float32)


def _bf16_split(a, bf16):
    hi = a.astype(bf16)
    lo = (a.astype(np.float32) - hi.astype(np.float32)).astype(bf16)
    return hi, lo


class _Builder:
    """Builds the per-core Bass program (identical on all cores; data differs)."""

    def __init__(self):
        sys.path.insert(0, '/opt/trn_rl_repo')
        import concourse.mybir as mybir
        from concourse import bass, tile
        from concourse.vector_clock import ScopedClock

        self.mybir = mybir
        self.bass = bass
        self.tile = tile
        self.ScopedClock = ScopedClock

    def make_tile_context(self, nc):
        mybir = self.mybir
        tile = self.tile
        ScopedClock = self.ScopedClock

        class PatchedTileContext(tile.TileContext):
            # walrus in this container rejects >1 sync-wait on the tail Drain
            # (setupSyncWait CTRL limit); spread the end-of-kernel waits
            # across single-wait SP nops instead.
            def _drain_and_barrier(self, tick_clock, wait_clock):
                nc_ = self.nc
                probe = nc_.sync.nop(nofuse=True)
                wait_clock.add_sem_waits(
                    probe.ins, ScopedClock({None: tick_clock.global_clock})
                )
                si = probe.ins.sync_info
                waits = list(si.on_wait) if si and si.on_wait else []
                if len(waits) > 1:
                    si.on_wait.clear()
                    si.on_wait.append(waits[0])
                    for w in waits[1:]:
                        n2 = nc_.sync.nop(nofuse=True)
                        s2 = n2.ins.sync_info
                        if s2 is None:
                            n2.ins.sync_info = mybir.SyncInfo(on_wait=[w], on_update=[])
                        else:
                            s2.on_wait.append(w)
                nc_.sync.drain()
                nc_.all_engine_barrier()
                assert self.sems is not None
                popped = nc_._tile_sem_poison_stack.pop()
                assert popped is self._sem_poison
                nc_.clear_and_free_semaphores(list(self.sems.allocated().values()))
                nc_.all_engine_barrier()

        return PatchedTileContext(nc)

    def build(self, inv_scale):
        mybir = self.mybir
        bass = self.bass
        dt = mybir.dt
        ALU = mybir.AluOpType
        F32, BF16, F16 = dt.float32, dt.bfloat16, dt.float16

        nc = bass.Bass(trn_type="TRN2", enable_partition_id=False)

        # ---- DRAM parameters ----
        # per-core labels, transposed-dense layout: row (d*16 + c) holds
        # labels[c*1024:(c+1)*1024, d]
        p_lab = nc.declare_dram_parameter("lab_td", [128, 1024], F32, isOutput=False)
        p_coef = nc.declare_dram_parameter("coeffs", [128, _DEG + 1], F32, isOutput=False)
        # affine stationary: rows 32i+0..5 = [w1h;w1h;w1l;w1l;1;1] for d=4g+i,
        # g selected by column block
        p_alhs = nc.declare_dram_parameter("aff_lhsT", [128, 256], BF16, isOutput=False)
        p_w2e = nc.declare_dram_parameter("w2eT", [128, 1024], BF16, isOutput=False)
        p_embw = nc.declare_dram_parameter("embw", [128, 128], BF16, isOutput=False)
        p_out = nc.declare_dram_parameter("outT", [128, _BC], F32, isOutput=True)

        from contextlib import ExitStack

        with self.make_tile_context(nc) as tc, ExitStack() as ctx:
            consts = ctx.enter_context(tc.tile_pool(name="consts", bufs=1))
            prep = ctx.enter_context(tc.tile_pool(name="prep", bufs=1))
            h1p = ctx.enter_context(tc.tile_pool(name="h1", bufs=2, space="PSUM"))
            pop = ctx.enter_context(tc.tile_pool(name="pout", bufs=2, space="PSUM"))
            sup = ctx.enter_context(tc.tile_pool(name="su", bufs=3))
            obp = ctx.enter_context(tc.tile_pool(name="ob", bufs=3))

            # ---- constants in ----
            t_coef = consts.tile([128, _DEG + 1], F32)
            nc.gpsimd.dma_start(t_coef[:], p_coef[:])
            t_alhs = consts.tile([128, 256], BF16)
            nc.gpsimd.dma_start(t_alhs[:], p_alhs[:])
            t_w2e = consts.tile([128, 1024], BF16)
            nc.gpsimd.dma_start(t_w2e[:], p_w2e[:])
            t_embw = consts.tile([128, 128], BF16)
            nc.gpsimd.dma_start(t_embw[:], p_embw[:])

            # ---- preamble: drop mask, safe labels, lncz polynomial ----
            t_x = prep.tile([128, 1024], F32)
            nc.gpsimd.dma_start(t_x[:], p_lab[:])

            t_eq = prep.tile([128, 1024], dt.uint8)
            nc.vector.tensor_tensor(t_eq[:], t_x[:], t_x[:], ALU.is_equal)

            t_safe = prep.tile([128, 1024], F32)
            nc.vector.memset(t_safe[:], 0.0)
            nc.vector.copy_predicated(t_safe[:], t_eq[:], t_x[:])

            t_u = prep.tile([128, 1024], F32)
            nc.vector.tensor_scalar_mul(t_u[:], t_safe[:], float(inv_scale))

            acc_a = prep.tile([128, 1024], F32)
            acc_b = prep.tile([128, 1024], F32)
            nc.vector.memset(acc_a[:], 0.0)
            cur, nxt = acc_a, acc_b
            for k in range(_DEG, 0, -1):
                nc.vector.scalar_tensor_tensor(
                    nxt[:], cur[:], t_coef[:, k:k + 1], t_u[:], ALU.add, ALU.mult
                )
                cur, nxt = nxt, cur
            t_lncz = prep.tile([128, 1024], F32)
            # lncz = poly + c0 where kept, -1e30 where dropped
            nc.vector.memset(t_lncz[:], -1.0e30)
            t_pl = nxt  # reuse the other ping-pong buffer
            nc.vector.tensor_scalar_add(t_pl[:], cur[:], t_coef[:, 0:1])
            nc.vector.copy_predicated(t_lncz[:], t_eq[:], t_pl[:])

            # bf16 hi/lo splits
            t_xh = prep.tile([128, 1024], BF16)
            nc.vector.tensor_copy(t_xh[:], t_safe[:])
            t_xl = prep.tile([128, 1024], BF16)
            nc.vector.tensor_tensor(t_xl[:], t_safe[:], t_xh[:], ALU.subtract)
            t_lh = prep.tile([128, 1024], BF16)
            nc.vector.tensor_copy(t_lh[:], t_lncz[:])
            t_ll = prep.tile([128, 1024], BF16)
            nc.vector.tensor_tensor(t_ll[:], t_lncz[:], t_lh[:], ALU.subtract)
            t_dropf = prep.tile([128, 1024], BF16)
            # drop = 1 - eq = eq * -1 + 1
            nc.vector.tensor_scalar(t_dropf[:], t_eq[:], -1.0, 1.0, ALU.mult, ALU.add)

            # ---- scatter into matmul-ready row layouts (SBUF->SBUF DMA) ----
            # Two column-halves per tensor so the first half's stripes can
            # start while the second half is still scattering.
            # afftile[g][h]: [128, BC/2] bf16, rows 32i+{0..5} =
            # [xh,xl,xh,xl,lh,ll] of d = 4g+i; batch b = c*1024 + h*512 + col.
            t_aff00 = consts.tile([128, _BC // 2], BF16)
            t_aff01 = consts.tile([128, _BC // 2], BF16)
            t_aff10 = consts.tile([128, _BC // 2], BF16)
            t_aff11 = consts.tile([128, _BC // 2], BF16)
            t_aff = [[t_aff00, t_aff01], [t_aff10, t_aff11]]
            t_emb0 = consts.tile([128, _BC // 2], BF16)
            t_emb1 = consts.tile([128, _BC // 2], BF16)
            t_emb = [t_emb0, t_emb1]
            rowsrc = [t_xh, t_xl, t_xh, t_xl, t_lh, t_ll]
            for h in range(2):
                csl = slice(512 * h, 512 * h + 512)
                for g in range(2):
                    for r, src in enumerate(rowsrc):
                        # in rows [64g:64g+64] iterate (i, c) lexicographic,
                        # matching out rows 32i+r (stride 32) x 16 col-blocks
                        nc.sync.dma_start(
                            t_aff[g][h][r:r + 97:32, :],
                            src[64 * g:64 * g + 64, csl],
                        )
                nc.sync.dma_start(t_emb[h][0:8, :], t_dropf[:, csl])

            # ---- main stripe loop (software-pipelined over d-groups) ----
            # Keep the PE stream dense (affine of group k+1 interleaves with
            # the exp-dependent mains of group k) so HAM stays at 2.4 GHz.
            Exp = mybir.ActivationFunctionType.Exp
            DGROUPS = [(0, 1, 2), (3, 4, 5), (6, 7)]
            NG = _NSTRIPE * 3

            def gslice(s):
                h, c = divmod(s, 16)
                return h, slice(512 * c, 512 * (c + 1))

            h1s = [None] * NG
            sus = [None] * NG

            def emit_aff(k):
                s, j = divmod(k, 3)
                h, sl = gslice(s)
                grp = DGROUPS[j]
                n = len(grp)
                h1 = h1p.tile([128, 512 * n], F32, tag="h1", name=f"h1_{k}")
                h1s[k] = h1
                for jj, d in enumerate(grp):
                    g, i = divmod(d, 4)
                    nc.tensor.matmul(
                        h1[:, 512 * jj:512 * (jj + 1)],
                        t_alhs[32 * i:32 * i + 6, 128 * g:128 * (g + 1)],
                        t_aff[g][h][32 * i:32 * i + 6, sl],
                        start=True, stop=True,
                        tile_position=(32 * i, 0),
                    )

            def emit_exp(k):
                s, j = divmod(k, 3)
                n = len(DGROUPS[j])
                su = sup.tile([128, 512 * n], BF16, tag="su", name=f"su_{k}")
                sus[k] = su
                nc.scalar.activation(su[:], h1s[k][:], Exp)

            pos = [None] * _NSTRIPE

            def emit_mains(k):
                s, j = divmod(k, 3)
                h, sl = gslice(s)
                grp = DGROUPS[j]
                if j == 0:
                    pos[s] = pop.tile([128, 512], F32, tag="po", name=f"po_{s}")
                po = pos[s]
                su = sus[k]
                for jj, d in enumerate(grp):
                    nc.tensor.matmul(
                        po[:],
                        t_w2e[:, 128 * d:128 * (d + 1)],
                        su[:, 512 * jj:512 * (jj + 1)],
                        start=(j == 0 and jj == 0), stop=False,
                    )
                if j == 2:
                    nc.tensor.matmul(
                        po[:], t_embw[0:8, :], t_emb[h][0:8, sl],
                        start=False, stop=True,
                    )
                    ob = obp.tile([128, 512], F32, tag="ob", name=f"ob_{s}")
                    nc.vector.tensor_copy(ob[:], po[:])
                    c = s % 16
                    osl = slice(1024 * c + 512 * (s // 16),
                                1024 * c + 512 * (s // 16) + 512)
                    nc.gpsimd.dma_start(p_out[:, osl], ob[:])

            # ---- HAM warm-up: ~5us of dense matmuls right before the
            # stripe loop (reads the scattered tile so it can't be scheduled
            # earlier). Gets the PE to 2.4 GHz; the steady-state gaps are too
            # short to re-throttle it.
            wtile = pop.tile([128, 512], F32, tag="po", name="warm")
            for it in range(16):
                nc.tensor.matmul(
                    wtile[:], t_alhs[0:6, 0:128], t_aff[0][0][0:6, 0:512],
                    start=True, stop=True, skip_group_check=True,
                    tile_position=(0, 0),
                )

            emit_aff(0)
            emit_exp(0)
            for k in range(NG):
                if k + 1 < NG:
                    emit_aff(k + 1)
                    emit_exp(k + 1)
                emit_mains(k)

        self._split_multi_waits(nc)
        return nc

    def _split_multi_waits(self, nc, maxw=1):
        """walrus in this container caps sync-waits per instruction at 2;
        move excess waits onto inserted same-engine NoOps."""
        mybir = self.mybir
        for f in nc.m.functions:
            for bb in f.blocks:
                new = []
                changed = False
                for ins in list(bb.instructions):
                    si = ins.sync_info
                    waits = list(si.on_wait) if si and si.on_wait else []
                    if len(waits) > maxw:
                        changed = True
                        extra, keep = waits[:-maxw], waits[-maxw:]
                        for j in range(0, len(extra), maxw):
                            new.append(mybir.InstNoOp(
                                name=f"{ins.name}_sw{j}", engine=ins.engine,
                                sync_info=mybir.SyncInfo(
                                    on_wait=list(extra[j:j + maxw]), on_update=[]),
                                text_hint="split_wait"))
                        si.on_wait.clear()
                        for w in keep:
                            si.on_wait.append(w)
                    new.append(ins)
                if changed:
                    bb.instructions = new


def _prepare_host(labels, emb_w, w1, b1, w2):
    import ml_dtypes
    bf16 = ml_dtypes.bfloat16

    S = float(max(6.0, np.nanmax(np.abs(labels)) * 1.02))
    coeffs = _fit_lnz_coeffs(w1, b1, S)  # [8, DEG+1] f32

    # coeff columns for the dense layout: partition p holds d = p // 16
    cc = np.zeros((128, _DEG + 1), np.float32)
    for p in range(128):
        cc[p] = coeffs[p // 16]

    w1h, w1l = _bf16_split(w1, bf16)            # [8, 128] each
    aff_lhsT = np.zeros((128, 256), bf16)
    ones = np.ones(_H, bf16)
    for d in range(_D):
        g, i = divmod(d, 4)
        rows = [w1h[d], w1h[d], w1l[d], w1l[d], ones, ones]
        for r, v in enumerate(rows):
            aff_lhsT[32 * i + r, 128 * g:128 * (g + 1)] = v

    w2e = (w2.astype(np.float64) * np.exp(b1.astype(np.float64))[:, None, :])
    w2eT = np.zeros((128, 1024), bf16)
    for d in range(_D):
        w2eT[:, 128 * d:128 * (d + 1)] = w2e[d].T.astype(bf16)

    embw = np.zeros((128, 128), bf16)
    embw[0:8] = emb_w.astype(bf16)

    # per-core transposed-dense labels: row 16*d + c = labels[c*1024:(c+1)*1024, d]
    lab_td = []
    for c in range(_NCORES):
        lc = labels[c * _BC:(c + 1) * _BC]               # [BC, 8]
        td = lc.reshape(16, 1024, 8).transpose(2, 0, 1).reshape(128, 1024)
        lab_td.append(np.ascontiguousarray(td, dtype=np.float32))

    const_map = {"coeffs": cc, "aff_lhsT": aff_lhsT, "w2eT": w2eT, "embw": embw}
    return S, lab_td, const_map


def _run_device(labels, emb_w, w1, b1, w2, trace=False):
    S, lab_td, const_map = _prepare_host(labels, emb_w, w1, b1, w2)
    builder = _Builder()
    nc = builder.build(1.0 / S)

    from concourse.bass_utils import run_bass_kernel_spmd
    in_maps = [{"lab_td": lab_td[c], **const_map} for c in range(_NCORES)]
    res = run_bass_kernel_spmd(
        nc, in_maps, list(range(_NCORES)), trace=trace
    )
    out = np.empty((_B, _H), np.float32)
    for c in range(_NCORES):
        out[c * _BC:(c + 1) * _BC] = res.results[c]["outT"].T
    return out, res


def kernel(labels, emb_w, w1, b1, w2, train, unconditioned):
    labels = np.asarray(labels)
    emb_w = np.asarray(emb_w, dtype=np.float32)
    w1 = np.asarray(w1, dtype=np.float32)
    b1 = np.asarray(b1, dtype=np.float32)
    w2 = np.asarray(w2, dtype=np.float32)
    if int(np.asarray(train)) or int(np.asarray(unconditioned)):
        return _np_reference(labels, emb_w, w1, b1, w2,
                             int(np.asarray(train)), int(np.asarray(unconditioned)))
    out, _ = _run_device(labels, emb_w, w1, b1, w2, trace=False)
    return out



# revision 4
# speedup vs baseline: 2.8280x; 2.8280x over previous
"""ConditionEmbedder kernel for 8 Trainium2 NeuronCores.

Math (train=0, unconditioned=0 path):
    drop = isnan(labels);  safe = where(drop, 0, labels)
    s    = softmax(safe[:,d,None]*w1[d] + b1[d], axis=-1)        # per (b, d)
    mlp  = s @ w2[d].T
    out  = sum_d where(drop, emb_w[d], mlp)                      # [B, H]

Algorithm: the per-(b,d) contribution mlp[b,d,:] is a smooth function
f_d: R -> R^128 of the single scalar labels[b,d].  Each component is fit
at runtime by a degree-13 Chebyshev polynomial on x in [-SFIT, SFIT]
(grid error ~4e-4 against an output scale of ~0.36).  The device then
computes, per sample,
    out[b] = sum_{d,p} c[d,p,:] * T_p(u_{b,d})  + c0_sum + emb fallback
as ONE K=112 fp16 matmul per 512-sample stripe: K = 8 dims x 13 Chebyshev
rows + 8 NaN-fallback indicator rows.  The T_p are built with the fp32
double-and-add recurrence (T_2n = 2 T_n^2 - 1, T_{m+n} = 2 T_m T_n -
T_{m-n}) spread across the vector/scalar/pool engines, converted to fp16,
and DMA-scattered into the (d,p)-partition matmul layout.  |x| > SFIT is
clamped to +-1 on device and those few samples (~0.2%) are patched
exactly on the host.  Output is written fp16 (adds < 2.5e-4 rel) and
upcast on the host, halving the dominant HBM write.

Device strategy (pure data parallel over batch, 8 cores): each core owns
B/8 = 16384 samples; weights/coefficients replicated; no collectives.
"""

import sys

import numpy as np

_B, _D, _H = 131072, 8, 128
_NCORES = 8
_BC = _B // _NCORES          # batch rows per core
_P = 13                      # Chebyshev degree
_SFIT = 3.0                  # fit half-range; |x| > _SFIT patched on host
_KV = _D * _P                # 104 polynomial rows
_K = _KV + _D                # + 8 drop-indicator rows = 112


def _np_reference(labels, emb_w, w1, b1, w2, train, unconditioned):
    """Slow exact fallback for the train/unconditioned branches (uses jax to
    reproduce the reference PRNG streams)."""
    import jax
    import jax.numpy as jnp

    DROPOUT_PROB = 0.1
    labels = jnp.asarray(labels)
    if unconditioned:
        drop = jnp.ones(labels.shape, dtype=bool)
    else:
        drop = jnp.isnan(labels)
        if train:
            rkey = jax.random.fold_in(jax.random.key(0), 1)
            drop = drop | (jax.random.uniform(rkey, labels.shape) < DROPOUT_PROB)
    safe = jnp.where(drop, 0.0, labels)
    h1 = safe[:, :, None] * w1[None, :, :] + b1[None, :, :]
    s = jax.nn.softmax(h1, axis=-1)
    mlp = jnp.einsum('bdh,dkh->bdk', s, w2)
    emb = jnp.where(drop[:, :, None], emb_w[None, :, :], mlp)
    if train:
        nkey = jax.random.fold_in(jax.random.key(0), 2)
        emb = emb + jax.random.normal(nkey, emb.shape, dtype=emb.dtype)
    return np.asarray(emb.sum(axis=1))


def _f_exact(x, w1d, b1d, w2d):
    """Exact f_d(x) = softmax(x*w1d + b1d) @ w2d.T, stable, fp64.
    x: [N]; returns [N, H]."""
    lg = x[:, None] * w1d[None, :].astype(np.float64) + b1d[None, :].astype(np.float64)
    m = lg.max(-1, keepdims=True)
    e = np.exp(lg - m)
    s = e / e.sum(-1, keepdims=True)
    return s @ w2d.astype(np.float64).T


def _fit_coeffs(w1, b1, w2):
    """Per-(d,k) Chebyshev coefficients on [-SFIT, SFIT], fp64 [D, P+1, H]."""
    import numpy.polynomial.chebyshev as CH

    G = 2049
    ug = -np.cos(np.linspace(0.0, np.pi, G))
    xg = _SFIT * ug
    Vmat = CH.chebvander(ug, _P)
    coefs = np.zeros((_D, _P + 1, _H))
    for d in range(_D):
        y = _f_exact(xg, w1[d], b1[d], w2[d])
        coefs[d] = np.linalg.lstsq(Vmat, y, rcond=None)[0]
    return coefs


class _Builder:
    """Builds the per-core Bass program (identical on all cores; data differs)."""

    def __init__(self):
        sys.path.insert(0, '/opt/trn_rl_repo')
        import concourse.mybir as mybir
        from concourse import bass, tile
        from concourse.vector_clock import ScopedClock

        self.mybir = mybir
        self.bass = bass
        self.tile = tile
        self.ScopedClock = ScopedClock

    def make_tile_context(self, nc):
        mybir = self.mybir
        tile = self.tile
        ScopedClock = self.ScopedClock

        class PatchedTileContext(tile.TileContext):
            # walrus in this container rejects >1 sync-wait on the tail Drain
            # (setupSyncWait CTRL limit); spread the end-of-kernel waits
            # across single-wait SP nops instead.
            def _drain_and_barrier(self, tick_clock, wait_clock):
                nc_ = self.nc
                probe = nc_.sync.nop(nofuse=True)
                wait_clock.add_sem_waits(
                    probe.ins, ScopedClock({None: tick_clock.global_clock})
                )
                si = probe.ins.sync_info
                waits = list(si.on_wait) if si and si.on_wait else []
                if len(waits) > 1:
                    si.on_wait.clear()
                    si.on_wait.append(waits[0])
                    for w in waits[1:]:
                        n2 = nc_.sync.nop(nofuse=True)
                        s2 = n2.ins.sync_info
                        if s2 is None:
                            n2.ins.sync_info = mybir.SyncInfo(on_wait=[w], on_update=[])
                        else:
                            s2.on_wait.append(w)
                nc_.sync.drain()
                nc_.all_engine_barrier()
                assert self.sems is not None
                popped = nc_._tile_sem_poison_stack.pop()
                assert popped is self._sem_poison
                nc_.clear_and_free_semaphores(list(self.sems.allocated().values()))
                nc_.all_engine_barrier()

        return PatchedTileContext(nc)

    def build(self, inv_scale):
        mybir = self.mybir
        bass = self.bass
        dt = mybir.dt
        ALU = mybir.AluOpType
        F32, F16, U8 = dt.float32, dt.float16, dt.uint8
        Copy = mybir.ActivationFunctionType.Copy
        Ident = mybir.ActivationFunctionType.Identity

        nc = bass.Bass(trn_type="TRN2", enable_partition_id=False)

        # ---- DRAM parameters ----
        # per-core labels, transposed-dense: row (16d + c) holds
        # labels[c*1024:(c+1)*1024, d] of this core's batch slice.
        p_lab = nc.declare_dram_parameter("lab_td", [128, 1024], F32, isOutput=False)
        # row 8(p-1)+d = cheb coeff c[d,p,:]; rows 104..111 = emb fallback
        p_lhsT = nc.declare_dram_parameter("lhsT", [_K, _H], F16, isOutput=False)
        p_c0 = nc.declare_dram_parameter("c0col", [128, 1], F32, isOutput=False)
        # transposed fp16 output; col 8192h + 512c + j = batch 1024c + 512h + j
        p_out = nc.declare_dram_parameter("outT", [128, _BC], F16, isOutput=True)

        from contextlib import ExitStack

        with self.make_tile_context(nc) as tc, ExitStack() as ctx:
            consts = ctx.enter_context(tc.tile_pool(name="consts", bufs=1))
            prep = ctx.enter_context(tc.tile_pool(name="prep", bufs=1))
            pop = ctx.enter_context(tc.tile_pool(name="po", bufs=2, space="PSUM"))
            obp = ctx.enter_context(tc.tile_pool(name="ob", bufs=3))

            t_lhsT = consts.tile([_K, _H], F16)
            nc.sync.dma_start(t_lhsT[:], p_lhsT[:])
            t_c0 = consts.tile([128, 1], F32)
            nc.sync.dma_start(t_c0[:], p_c0[:])
            t_lab = consts.tile([128, 1024], F32)
            nc.sync.dma_start(t_lab[:], p_lab[:])

            # matmul rhs per half: rows 8(p-1)+d = T_p(u_d); 104..111 = dropf
            t_R = [consts.tile([_K, _BC // 2], F16, name=f"R{h}") for h in (0, 1)]

            vec, act, pool = nc.vector, nc.scalar, nc.gpsimd

            def preamble(h):
                sl = slice(512 * h, 512 * h + 512)
                cnt = [0]

                def tl(dtype=F32):
                    cnt[0] += 1
                    return prep.tile([128, 512], dtype, name=f"t{h}_{cnt[0]}")

                eq = prep.tile([128, 512], U8, name=f"eq{h}")
                vec.tensor_tensor(eq[:], t_lab[:, sl], t_lab[:, sl], ALU.is_equal)
                safe = tl()
                pool.memset(safe[:], 0.0)
                vec.copy_predicated(safe[:], eq[:], t_lab[:, sl])
                us = tl()
                act.activation(us[:], safe[:], Copy, scale=float(inv_scale))

                T = {}
                T[1] = tl()
                vec.tensor_scalar(T[1][:], us[:], -1.0, 1.0, ALU.max, ALU.min)

                V = {p: tl(F16) for p in range(1, _P + 1)}
                act.copy(V[1][:], T[1][:])

                # dropf = 1 - eq  (fp16 rows for the fallback matmul)
                vdrop = tl(F16)
                pool.tensor_scalar(vdrop[:], eq[:], -1.0, 1.0, ALU.mult, ALU.add)

                def even(n, out16=None):
                    # T_2n = 2*T_n^2 - 1
                    s = tl()
                    act.square(s[:], T[n][:])
                    if out16 is None:
                        T[2 * n] = tl()
                        pool.tensor_scalar(T[2 * n][:], s[:], 2.0, -1.0,
                                           ALU.mult, ALU.add)
                    else:
                        pool.tensor_scalar(out16[:], s[:], 2.0, -1.0,
                                           ALU.mult, ALU.add)

                def odd(n, out16=None):
                    # T_{2n+1} = 2*T_n*T_{n+1} - T_1
                    m = tl()
                    vec.tensor_tensor(m[:], T[n][:], T[n + 1][:], ALU.mult)
                    if out16 is None:
                        T[2 * n + 1] = tl()
                        vec.scalar_tensor_tensor(T[2 * n + 1][:], m[:], 2.0,
                                                 T[1][:], ALU.mult, ALU.subtract)
                    else:
                        vec.scalar_tensor_tensor(out16[:], m[:], 2.0,
                                                 T[1][:], ALU.mult, ALU.subtract)

                even(1)            # T2
                odd(1)             # T3
                even(2)            # T4
                odd(2)             # T5
                even(3)            # T6
                odd(3)             # T7
                pool.tensor_copy(V[2][:], T[2][:])
                act.copy(V[3][:], T[3][:])
                pool.tensor_copy(V[4][:], T[4][:])
                act.copy(V[5][:], T[5][:])
                pool.tensor_copy(V[6][:], T[6][:])
                act.copy(V[7][:], T[7][:])
                even(4, V[8])      # T8..T13 only needed in fp16
                odd(4, V[9])
                even(5, V[10])
                odd(5, V[11])
                even(6, V[12])
                odd(6, V[13])

                # scatter into the (p,d)-partition matmul layout.
                # dst rows 8(p-1)..8(p-1)+7 iterate d, then free (c, j);
                # src [128, 512] iterates partitions (d, c), then j: match.
                R = t_R[h]
                for p in range(1, _P + 1):
                    nc.sync.dma_start(R[8 * (p - 1):8 * p, :], V[p][:])
                nc.sync.dma_start(R[_KV:_KV + 8, :], vdrop[:])

            for h in (0, 1):
                preamble(h)

            # ---- stripe loop: one K=112 matmul per 512 samples ----
            # (GPSIMD cannot read PSUM: copies go on scalar/vector only,
            # 2 stripes per copy to halve instruction count.)
            for h in (0, 1):
                for q in (0, 1):
                    ob = obp.tile([128, 4096], F16, tag="ob", name=f"ob{h}{q}")
                    for g in range(4):
                        po = pop.tile([128, 1024], F32, tag="po", name=f"po{h}{q}{g}")
                        for j in (0, 1):
                            c = 8 * q + 2 * g + j
                            nc.tensor.matmul(
                                po[:, 512 * j:512 * (j + 1)], t_lhsT[:, :],
                                t_R[h][:, 512 * c:512 * (c + 1)],
                                start=True, stop=True,
                            )
                        dst = ob[:, 1024 * g:1024 * (g + 1)]
                        if g % 2 == 0:
                            act.activation(dst, po[:], Ident, bias=t_c0[:, 0:1])
                        else:
                            vec.tensor_scalar_add(dst, po[:], t_c0[:, 0:1])
                    osl = slice(8192 * h + 4096 * q, 8192 * h + 4096 * (q + 1))
                    nc.scalar.dma_start(p_out[:, osl], ob[:])

        self._split_multi_waits(nc)
        return nc

    def _split_multi_waits(self, nc, maxw=1):
        """walrus in this container caps sync-waits per instruction at 2;
        move excess waits onto inserted same-engine NoOps."""
        mybir = self.mybir
        for f in nc.m.functions:
            for bb in f.blocks:
                new = []
                changed = False
                for ins in list(bb.instructions):
                    si = ins.sync_info
                    waits = list(si.on_wait) if si and si.on_wait else []
                    if len(waits) > maxw:
                        changed = True
                        extra, keep = waits[:-maxw], waits[-maxw:]
                        for j in range(0, len(extra), maxw):
                            new.append(mybir.InstNoOp(
                                name=f"{ins.name}_sw{j}", engine=ins.engine,
                                sync_info=mybir.SyncInfo(
                                    on_wait=list(extra[j:j + maxw]), on_update=[]),
                                text_hint="split_wait"))
                        si.on_wait.clear()
                        for w in keep:
                            si.on_wait.append(w)
                    new.append(ins)
                if changed:
                    bb.instructions = new


def _prepare_host(labels, emb_w, w1, b1, w2):
    fp16 = np.float16
    coefs = _fit_coeffs(w1, b1, w2)              # [D, P+1, H] fp64
    c16 = coefs[:, 1:, :].astype(fp16)           # [D, P, H]
    c0col = coefs[:, 0, :].sum(0).astype(np.float32).reshape(128, 1)

    # device value at u=0 (dropped entries): c0 + sum_p c16[d,p]*T_p(0)
    Tp0 = np.cos(np.arange(1, _P + 1) * np.pi / 2.0)
    Tp0[np.abs(Tp0) < 0.5] = 0.0
    poly0 = coefs[:, 0, :] + np.einsum('p,dpk->dk', Tp0, c16.astype(np.float64))
    emb_corr = (emb_w.astype(np.float64) - poly0).astype(fp16)   # [D, H]

    lhsT = np.zeros((_K, _H), fp16)
    for d in range(_D):
        for p in range(1, _P + 1):
            lhsT[8 * (p - 1) + d] = c16[d, p - 1]
        lhsT[_KV + d] = emb_corr[d]

    # per-core transposed-dense labels: row 16d + c = labels[c*1024:(c+1)*1024, d]
    lab_td = []
    for cc in range(_NCORES):
        lc = labels[cc * _BC:(cc + 1) * _BC]             # [BC, 8]
        td = lc.reshape(16, 1024, 8).transpose(2, 0, 1).reshape(128, 1024)
        lab_td.append(np.ascontiguousarray(td, dtype=np.float32))

    const_map = {"lhsT": lhsT, "c0col": c0col}
    return lab_td, const_map, coefs, c16


def _patch_host(out, labels, w1, b1, w2, coefs, c16):
    """Exactly fix samples where |x| > SFIT (device clamped u to +-1)."""
    flag = np.isfinite(labels) & (np.abs(labels) > _SFIT)
    if not flag.any():
        return
    bb, dd = np.nonzero(flag)
    xv = labels[bb, dd].astype(np.float64)
    sgn = np.sign(xv)
    fex = np.empty((len(bb), _H))
    for d in np.unique(dd):
        m = dd == d
        fex[m] = _f_exact(xv[m], w1[d], b1[d], w2[d])
    pw = sgn[:, None] ** np.arange(1, _P + 1)[None, :]          # [N, P]
    dev = coefs[dd, 0, :] + np.einsum('np,npk->nk', pw, c16[dd].astype(np.float64))
    np.add.at(out, bb, (fex - dev).astype(np.float32))


def _run_device(labels, emb_w, w1, b1, w2, trace=False):
    lab_td, const_map, coefs, c16 = _prepare_host(labels, emb_w, w1, b1, w2)
    builder = _Builder()
    nc = builder.build(1.0 / _SFIT)

    from concourse.bass_utils import run_bass_kernel_spmd
    in_maps = [{"lab_td": lab_td[c], **const_map} for c in range(_NCORES)]
    res = run_bass_kernel_spmd(
        nc, in_maps, list(range(_NCORES)), trace=trace
    )
    out = np.empty((_B, _H), np.float32)
    for c in range(_NCORES):
        v = res.results[c]["outT"].reshape(128, 2, 16, 512)
        out[c * _BC:(c + 1) * _BC] = (
            v.transpose(2, 1, 3, 0).reshape(_BC, 128).astype(np.float32))
    _patch_host(out, labels, w1, b1, w2, coefs, c16)
    return out, res


def kernel(labels, emb_w, w1, b1, w2, train, unconditioned):
    labels = np.asarray(labels)
    emb_w = np.asarray(emb_w, dtype=np.float32)
    w1 = np.asarray(w1, dtype=np.float32)
    b1 = np.asarray(b1, dtype=np.float32)
    w2 = np.asarray(w2, dtype=np.float32)
    if int(np.asarray(train)) or int(np.asarray(unconditioned)):
        return _np_reference(labels, emb_w, w1, b1, w2,
                             int(np.asarray(train)), int(np.asarray(unconditioned)))
    out, _ = _run_device(labels, emb_w, w1, b1, w2, trace=False)
    return out
